# revision 2
# baseline (speedup 1.0000x reference)
"""Trainium2 Bass kernel for nn_Decoder (image-caption LSTM decoder).

Model (per reference):
  sort batch by caption length desc; feats_mean = mean over patches;
  h0 = feats_mean @ att1_w.T + att1_b; c0 = 0;
  LSTM over T=L-1 steps with embedded (sorted) captions as inputs;
  per-step vocab projection pred_t = (h_t @ word_w.T + word_b) * mask_t.

Sharding: data-parallel over the batch. Sorted rows are dealt round-robin
(core c takes sorted rows c::8) so the ragged tail is balanced. Each core
runs the full pipeline for its 16 rows; no collectives. Host does only
permutation/layout prep and final gather.

Device pipeline per core (all compute on NeuronCore):
  P1: stream image features, mean over patches via ones-block matmul,
      h0 = mean @ att1T (+bias), transposed into state layout.
  P2: dma_gather embedding rows (t-major), PE-transpose, XW = emb @ w_ihT
      + (b_ih+b_hh) staged to a DRAM scratch (SBUF is too small for XW).
  P3: 51 LSTM steps. Gates psum accumulates an identity-matmul injection
      of XW_t plus h_{t-1} @ w_hhT; sigmoid/tanh on ACT; cell math on DVE;
      h_t is PE-transposed into an SBUF history hTh[K, D/128, t*16+b]
      which is exactly the lhsT layout the word matmul needs.
  P4: vocab projection: out[rows, V] = hTh.T @ word_wT streamed by
      512-wide vocab chunks, masked by a per-row scalar, DMA'd out.
"""

import functools

import numpy as np

import concourse.bacc as bacc
import concourse.bass as bass
import concourse.tile as tile
from concourse import mybir

F32 = mybir.dt.float32
I16 = mybir.dt.int16


def cdiv(a, b):
    return -(-a // b)


# ---------------------------------------------------------------------------
# problem configuration (hardcoded for the graded problem)
# ---------------------------------------------------------------------------
N_CORES = 8
REAL_CFG = dict(B=128, BL=16, T=51, NPATCH=196, FEAT=2048, E=512, D=512, V=10000)


def derived(cfg):
    d = dict(cfg)
    BL, T = d["BL"], d["T"]
    d["G4"] = 4 * d["D"]
    d["ROWS"] = BL * T
    d["MCH"] = cdiv(d["ROWS"], 128)
    d["ROWS_PAD"] = d["MCH"] * 128
    d["KD"] = d["D"] // 128
    d["KE"] = d["E"] // 128
    d["KF"] = d["FEAT"] // 128
    d["BR"] = BL * d["NPATCH"]
    d["KB"] = cdiv(d["BR"], 128)
    d["NCH"] = cdiv(d["G4"], 512)
    d["NV"] = cdiv(d["V"], 512)
    d["S"] = cdiv(d["ROWS"], 16)
    return d


# ---------------------------------------------------------------------------
# kernel builder (shape-parameterized; one NeuronCore's program)
# ---------------------------------------------------------------------------
def build_decoder_kernel(tc, ins, outs, cfg, has_word_bias=False):
    nc = tc.nc
    c = derived(cfg)
    BL, T, NPATCH, FEAT, E, D, V = (
        c["BL"], c["T"], c["NPATCH"], c["FEAT"], c["E"], c["D"], c["V"])
    G4, ROWS, MCH, ROWS_PAD = c["G4"], c["ROWS"], c["MCH"], c["ROWS_PAD"]
    KD, KE, KF, BR, KB, NCH, NV = (
        c["KD"], c["KE"], c["KF"], c["BR"], c["KB"], c["NCH"], c["NV"])
    assert D % 128 == 0 and E % 128 == 0 and FEAT % 512 == 0 and G4 % 512 == 0
    assert BL <= 128 and (E * 4) % 256 == 0

    xw_dram = nc.dram_tensor("xw_scratch", [ROWS_PAD, G4], F32, kind="Internal").ap()

    import contextlib
    ctx = contextlib.ExitStack()
    with ctx:
        singles = ctx.enter_context(tc.tile_pool(name="singles", bufs=1))
        s8 = ctx.enter_context(tc.tile_pool(name="s8", bufs=3))
        xwp = ctx.enter_context(tc.tile_pool(name="xwp", bufs=3))
        actp = ctx.enter_context(tc.tile_pool(name="actp", bufs=2))
        cellp = ctx.enter_context(tc.tile_pool(name="cellp", bufs=2))
        outp = ctx.enter_context(tc.tile_pool(name="outp", bufs=4))
        psum = ctx.enter_context(tc.tile_pool(name="psum", bufs=1, space="PSUM"))
        pstr = ctx.enter_context(tc.tile_pool(name="pstr", bufs=2, space="PSUM"))
        psio = ctx.enter_context(tc.tile_pool(name="psio", bufs=2, space="PSUM"))

        # ---- persistent tiles -------------------------------------------
        ident_sb = singles.tile([128, 128], F32)
        nc.sync.dma_start(out=ident_sb, in_=ins["ident"])
        whh_sb = singles.tile([128, KD, G4], F32)
        nc.sync.dma_start(
            out=whh_sb, in_=ins["whhT"].rearrange("(k p) g -> p k g", p=128))
        biasg_sb = singles.tile([128, G4], F32)
        bg = ins["biasg"]
        nc.sync.dma_start(
            out=biasg_sb,
            in_=bass.AP(tensor=bg.tensor, offset=bg.offset,
                        ap=[[0, 128], bg.ap[-1]]))
        mask_sb = singles.tile([128, MCH], F32)
        nc.sync.dma_start(out=mask_sb, in_=ins["mask"])
        ones_sb = singles.tile([128, KB, BL], F32)
        nc.sync.dma_start(
            out=ones_sb, in_=ins["ones"].rearrange("(k p) b -> p k b", p=128))
        attb_sb = singles.tile([BL, D], F32)
        ab = ins["att1b"]
        nc.sync.dma_start(
            out=attb_sb,
            in_=bass.AP(tensor=ab.tensor, offset=ab.offset,
                        ap=[[0, BL], ab.ap[-1]]))
        idxs_sb = singles.tile([128, c["S"]], I16)
        nc.sync.dma_start(out=idxs_sb, in_=ins["idx"])
        hTh = singles.tile([128, KD, ROWS], F32)
        hT0 = singles.tile([128, KD, BL], F32)
        embT = singles.tile([128, KE, ROWS_PAD], F32)
        meanT = singles.tile([128, KF, BL], F32)
        emb_sb = singles.tile([128, MCH, E], F32)

        # ---- P1: mean over patches -> h0 --------------------------------
        ps_mean = psum.tile([BL, max(G4, FEAT)], F32, tag="gates")
        for kb in range(KB):
            rk = min(128, BR - kb * 128)
            ft = s8.tile([128, FEAT], F32, tag="s8", name="ft")
            nc.sync.dma_start(out=ft[:rk], in_=ins["feats"][kb * 128:kb * 128 + rk, :])
            for nf in range(FEAT // 512):
                nc.tensor.matmul(
                    ps_mean[:, nf * 512:(nf + 1) * 512],
                    lhsT=ones_sb[:rk, kb, :],
                    rhs=ft[:rk, nf * 512:(nf + 1) * 512],
                    start=(kb == 0), stop=(kb == KB - 1))
        mean_sb = actp.tile([BL, max(G4, FEAT)], F32, tag="act", name="mean_sb")
        nc.scalar.activation(
            mean_sb[:, :FEAT], ps_mean[:, :FEAT],
            mybir.ActivationFunctionType.Copy, scale=1.0 / NPATCH)
        # transpose mean -> meanT
        ps_mt = pstr.tile([128, KF, BL], F32, tag="tr", name="ps_mt")
        for kf in range(KF):
            nc.tensor.transpose(
                ps_mt[:, kf, :], mean_sb[:BL, kf * 128:(kf + 1) * 128],
                ident_sb[:BL, :BL])
        nc.scalar.copy(meanT, ps_mt)
        # h0 = mean @ att1T + att1b
        ps_h0 = psio.tile([128, 512], F32, tag="io", name="ps_h0")
        for kf in range(KF):
            at = s8.tile([128, D], F32, tag="s8", name="at")
            nc.sync.dma_start(
                out=at, in_=ins["att1T"].rearrange("(k p) d -> p k d", p=128)[:, kf, :])
            nc.tensor.matmul(ps_h0[:BL, :D], lhsT=meanT[:, kf, :], rhs=at,
                             start=(kf == 0), stop=(kf == KF - 1))
        h0_sb = cellp.tile([BL, D], F32, tag="h", name="h0_sb")
        nc.vector.tensor_add(h0_sb, ps_h0[:BL, :D], attb_sb)
        ps_t0 = pstr.tile([128, KD, BL], F32, tag="tr", name="ps_t0")
        for kd in range(KD):
            nc.tensor.transpose(
                ps_t0[:, kd, :], h0_sb[:, kd * 128:(kd + 1) * 128],
                ident_sb[:BL, :BL])
        nc.scalar.copy(hT0, ps_t0)

        # ---- P2: embedding gather + XW precompute -----------------------
        nc.vector.memset(emb_sb, 0.0)
        nc.gpsimd.dma_gather(
            out_ap=emb_sb, in_ap=ins["emb"], idxs_ap=idxs_sb,
            num_idxs=ROWS, num_idxs_reg=ROWS, elem_size=E)
        for m in range(MCH):
            for ke in range(KE):
                ps_e = pstr.tile([128, 128], F32, tag="tr", name="ps_e")
                nc.tensor.transpose(
                    ps_e, emb_sb[:, m, ke * 128:(ke + 1) * 128], ident_sb)
                if (m + ke) % 2 == 0:
                    nc.scalar.copy(embT[:, ke, m * 128:(m + 1) * 128], ps_e)
                else:
                    nc.vector.tensor_copy(embT[:, ke, m * 128:(m + 1) * 128], ps_e)
        for nch in range(NCH):
            gs = slice(nch * 512, (nch + 1) * 512)
            wi = s8.tile([128, KE, 512], F32, tag="s8", name="wi")
            nc.sync.dma_start(
                out=wi,
                in_=ins["wihT"].rearrange("(k p) g -> p k g", p=128)[:, :, gs])
            for m in range(MCH):
                mr = min(128, ROWS - m * 128)
                ps_xw = psio.tile([128, 512], F32, tag="io", name="ps_xw")
                for ke in range(KE):
                    nc.tensor.matmul(
                        ps_xw[:mr], lhsT=embT[:, ke, m * 128:m * 128 + mr],
                        rhs=wi[:, ke, :], start=(ke == 0), stop=(ke == KE - 1))
                xo = outp.tile([128, 512], F32, tag="out", name="xo")
                nc.vector.tensor_add(xo[:mr], ps_xw[:mr], biasg_sb[:mr, gs])
                nc.sync.dma_start(
                    out=xw_dram[m * 128:m * 128 + mr, gs], in_=xo[:mr])

        # ---- P3: LSTM recurrence ----------------------------------------
        c_prev = cellp.tile([BL, D], F32, tag="c", name="c_init")
        nc.vector.memset(c_prev, 0.0)
        h_prev = hT0
        for t in range(T):
            xw_sb = xwp.tile([BL, G4], F32, tag="xw", name="xw_sb")
            nc.sync.dma_start(out=xw_sb, in_=xw_dram[t * BL:(t + 1) * BL, :])
            ps_g = psum.tile([BL, max(G4, FEAT)], F32, tag="gates", name="ps_g")
            for nch in range(NCH):
                gs = slice(nch * 512, (nch + 1) * 512)
                nc.tensor.matmul(ps_g[:, gs], lhsT=ident_sb[:BL, :BL],
                                 rhs=xw_sb[:, gs], start=True, stop=False)
            for kd in range(KD):
                for nch in range(NCH):
                    gs = slice(nch * 512, (nch + 1) * 512)
                    nc.tensor.matmul(
                        ps_g[:, gs], lhsT=h_prev[:, kd, :], rhs=whh_sb[:, kd, gs],
                        start=False, stop=(kd == KD - 1))
            act_sb = actp.tile([BL, max(G4, FEAT)], F32, tag="act", name="act_sb")
            nc.scalar.activation(act_sb[:, 0:2 * D], ps_g[:, 0:2 * D],
                                 mybir.ActivationFunctionType.Sigmoid)
            nc.scalar.activation(act_sb[:, 2 * D:3 * D], ps_g[:, 2 * D:3 * D],
                                 mybir.ActivationFunctionType.Tanh)
            nc.scalar.activation(act_sb[:, 3 * D:4 * D], ps_g[:, 3 * D:4 * D],
                                 mybir.ActivationFunctionType.Sigmoid)
            ig = cellp.tile([BL, D], F32, tag="ig", name="ig")
            nc.vector.tensor_mul(ig, act_sb[:, 0:D], act_sb[:, 2 * D:3 * D])
            fc = cellp.tile([BL, D], F32, tag="fc", name="fc")
            nc.vector.tensor_mul(fc, act_sb[:, D:2 * D], c_prev)
            c_new = cellp.tile([BL, D], F32, tag="c", name="c_new")
            nc.vector.tensor_add(c_new, fc, ig)
            tanhc = cellp.tile([BL, D], F32, tag="th", name="tanhc")
            nc.scalar.activation(tanhc, c_new, mybir.ActivationFunctionType.Tanh)
            h_sb = cellp.tile([BL, D], F32, tag="h", name="h_sb")
            nc.vector.tensor_mul(h_sb, act_sb[:, 3 * D:4 * D], tanhc)
            ps_tr = pstr.tile([128, KD, BL], F32, tag="tr", name="ps_tr")
            for kd in range(KD):
                nc.tensor.transpose(
                    ps_tr[:, kd, :], h_sb[:, kd * 128:(kd + 1) * 128],
                    ident_sb[:BL, :BL])
            nc.scalar.copy(hTh[:, :, t * BL:(t + 1) * BL], ps_tr)
            c_prev = c_new
            h_prev = hTh[:, :, t * BL:(t + 1) * BL]

        # ---- P4: vocab projection + mask + store ------------------------
        for nv in range(NV):
            nvs = min(512, V - nv * 512)
            wv = s8.tile([128, KD, 512], F32, tag="s8", name="wv")
            nc.sync.dma_start(
                out=wv[:, :, :nvs],
                in_=ins["wordT"].rearrange("(k p) v -> p k v", p=128)
                [:, :, nv * 512:nv * 512 + nvs])
            if has_word_bias:
                wb = ins["wordb"]
                wb_sb = outp.tile([128, 512], F32, tag="wb", name="wb_sb")
                nc.sync.dma_start(
                    out=wb_sb[:, :nvs],
                    in_=bass.AP(tensor=wb.tensor,
                                offset=wb.offset + nv * 512,
                                ap=[[0, 128], [1, nvs]]))
            for m in range(MCH):
                mr = min(128, ROWS - m * 128)
                ps_w = psio.tile([128, 512], F32, tag="io", name="ps_w")
                for kd in range(KD):
                    nc.tensor.matmul(
                        ps_w[:mr, :nvs], lhsT=hTh[:, kd, m * 128:m * 128 + mr],
                        rhs=wv[:, kd, :nvs], start=(kd == 0), stop=(kd == KD - 1))
                ou = outp.tile([128, 512], F32, tag="out", name="ou")
                if has_word_bias:
                    nc.vector.tensor_add(ou[:mr, :nvs], ps_w[:mr, :nvs],
                                         wb_sb[:mr, :nvs])
                    nc.vector.tensor_scalar_mul(ou[:mr, :nvs], ou[:mr, :nvs],
                                                mask_sb[:mr, m:m + 1])
                elif (m + nv) % 2 == 0:
                    nc.scalar.activation(
                        ou[:mr, :nvs], ps_w[:mr, :nvs],
                        mybir.ActivationFunctionType.Copy,
                        scale=mask_sb[:mr, m:m + 1])
                else:
                    nc.vector.tensor_scalar_mul(ou[:mr, :nvs], ps_w[:mr, :nvs],
                                                mask_sb[:mr, m:m + 1])
                nc.sync.dma_start(
                    out=outs["preds"][m * 128:m * 128 + mr,
                                      nv * 512:nv * 512 + nvs],
                    in_=ou[:mr, :nvs])


# ---------------------------------------------------------------------------
# host-side input prep
# ---------------------------------------------------------------------------
def prep_core_inputs(cfg, rows_feats, rows_caps, rows_declen, shared):
    """Build the per-core in_map. rows_* are this core's (sorted) slices."""
    c = derived(cfg)
    BL, T = c["BL"], c["T"]
    m = dict(shared)
    m["feats"] = np.ascontiguousarray(
        rows_feats.reshape(c["BR"], c["FEAT"]).astype(np.float32))
    idx_flat = np.ascontiguousarray(rows_caps[:, :T].T).reshape(-1)
    pad = c["S"] * 16 - idx_flat.shape[0]
    idx_flat = np.concatenate([idx_flat, -np.ones(pad, np.int64)])
    m["idx"] = np.ascontiguousarray(
        np.tile(idx_flat.reshape(c["S"], 16).T, (8, 1)).astype(np.int16))
    mk = (np.arange(T)[:, None] < rows_declen[None, :]).astype(np.float32)
    mk = np.concatenate([mk.reshape(-1),
                         np.zeros(c["ROWS_PAD"] - c["ROWS"], np.float32)])
    m["mask"] = np.ascontiguousarray(mk.reshape(c["MCH"], 128).T)
    return m


def prep_shared_inputs(cfg, emb_table, att1_w, att1_b, w_ih, w_hh, b_ih, b_hh,
                       word_w, word_b):
    c = derived(cfg)
    f32 = np.float32
    shared = {
        "emb": np.ascontiguousarray(emb_table, f32),
        "wihT": np.ascontiguousarray(w_ih.T, f32),
        "whhT": np.ascontiguousarray(w_hh.T, f32),
        "biasg": (b_ih + b_hh).astype(f32),
        "att1T": np.ascontiguousarray(att1_w.T, f32),
        "att1b": np.ascontiguousarray(att1_b, f32),
        "wordT": np.ascontiguousarray(word_w.T, f32),
        "ident": np.eye(128, dtype=f32),
    }
    ones = np.zeros((c["KB"] * 128, c["BL"]), f32)
    for b in range(c["BL"]):
        ones[b * c["NPATCH"]:(b + 1) * c["NPATCH"], b] = 1.0
    shared["ones"] = ones
    if np.any(word_b):
        shared["wordb"] = np.ascontiguousarray(word_b, f32)
    return shared


def declare_io(nc, cfg, has_word_bias):
    c = derived(cfg)
    def di(name, shape, dt=F32):
        return nc.dram_tensor(name, list(shape), dt, kind="ExternalInput").ap()
    ins = {
        "feats": di("feats", (c["BR"], c["FEAT"])),
        "emb": di("emb", (c["V"], c["E"])),
        "idx": di("idx", (128, c["S"]), I16),
        "wihT": di("wihT", (c["E"], c["G4"])),
        "whhT": di("whhT", (c["D"], c["G4"])),
        "biasg": di("biasg", (c["G4"],)),
        "att1T": di("att1T", (c["FEAT"], c["D"])),
        "att1b": di("att1b", (c["D"],)),
        "wordT": di("wordT", (c["D"], c["V"])),
        "ident": di("ident", (128, 128)),
        "ones": di("ones", (c["KB"] * 128, c["BL"])),
        "mask": di("mask", (128, c["MCH"])),
    }
    if has_word_bias:
        ins["wordb"] = di("wordb", (c["V"],))
    outs = {
        "preds": nc.dram_tensor(
            "preds", [c["ROWS"], c["V"]], F32, kind="ExternalOutput").ap(),
    }
    return ins, outs


@functools.lru_cache(maxsize=2)
def _get_compiled(has_word_bias):
    cfg = REAL_CFG
    nc = bacc.Bacc("TRN2", target_bir_lowering=False, debug=False,
                   enable_asserts=False, num_devices=N_CORES)
    ins, outs = declare_io(nc, cfg, has_word_bias)
    with tile.TileContext(nc) as tc:
        build_decoder_kernel(tc, ins, outs, cfg, has_word_bias)
    nc.compile()
    return nc


# ---------------------------------------------------------------------------
# public entry point
# ---------------------------------------------------------------------------
def kernel(image_features, encoded_captions, caption_lengths, emb_table,
           att1_w, att1_b, w_ih, w_hh, b_ih, b_hh, word_w, word_b,
           _trace=False, _trace_kwargs=None):
    from concourse import bass_utils

    cfg = REAL_CFG
    c = derived(cfg)
    B, BL, T, V = cfg["B"], cfg["BL"], cfg["T"], cfg["V"]

    lens = np.asarray(caption_lengths)[:, 0]
    sort_ind = np.argsort(-lens.astype(np.int64), kind="stable")
    lens_s = lens[sort_ind]
    caps = np.asarray(encoded_captions)[sort_ind]
    dec_len = (lens_s - 1).astype(caption_lengths.dtype)
    idx_dtype = np.int64 if caption_lengths.dtype == np.int64 else np.int32
    sort_ind = sort_ind.astype(idx_dtype)

    has_wb = bool(np.any(np.asarray(word_b)))
    shared = prep_shared_inputs(cfg, emb_table, att1_w, att1_b, w_ih, w_hh,
                                b_ih, b_hh, word_w, word_b)
    feats_np = np.asarray(image_features)
    in_maps = []
    for cid in range(N_CORES):
        rows = sort_ind[cid::N_CORES].astype(np.int64)
        in_maps.append(prep_core_inputs(
            cfg, feats_np[rows], caps[cid::N_CORES],
            np.asarray(dec_len)[cid::N_CORES].astype(np.int64), shared))

    nc = _get_compiled(has_wb)
    res = bass_utils.run_bass_kernel_spmd(
        nc, in_maps, list(range(N_CORES)), trace=_trace,
        **(_trace_kwargs or {}))

    predictions = np.empty((B, T, V), np.float32)
    for cid in range(N_CORES):
        p = res.results[cid]["preds"].reshape(T, BL, V)
        predictions[cid::N_CORES] = np.swapaxes(p, 0, 1)
    kernel.last_results = res
    return predictions, caps, dec_len, sort_ind


# revision 4
# speedup vs baseline: 2.2569x; 2.2569x over previous
"""Trainium2 Bass kernel for nn_Decoder (image-caption LSTM decoder).

Model (per reference):
  sort batch by caption length desc; feats_mean = mean over patches;
  h0 = feats_mean @ att1_w.T + att1_b; c0 = 0;
  LSTM over T=L-1 steps with embedded (sorted) captions as inputs;
  per-step vocab projection pred_t = (h_t @ word_w.T + word_b) * mask_t.

Sharding: data-parallel over the batch. Sorted rows are dealt round-robin
(core c takes sorted rows c::8) so the ragged tail is balanced. Each core
runs the full pipeline for its 16 rows; no collectives. Host does only
permutation/layout prep and final gather.

Device pipeline per core (all compute on NeuronCore):
  P1: stream image features, mean over patches via ones-block matmul,
      h0 = mean @ att1T (+bias), transposed into state layout.
  P2: dma_gather embedding rows (t-major), PE-transpose, XW = emb @ w_ihT
      + (b_ih+b_hh) staged to a DRAM scratch (SBUF is too small for XW).
  P3: 51 LSTM steps. Gates psum accumulates an identity-matmul injection
      of XW_t plus h_{t-1} @ w_hhT; sigmoid/tanh on ACT; cell math on DVE;
      h_t is PE-transposed into an SBUF history hTh[K, D/128, t*16+b]
      which is exactly the lhsT layout the word matmul needs.
  P4: vocab projection: out[rows, V] = hTh.T @ word_wT streamed by
      512-wide vocab chunks, masked by a per-row scalar, DMA'd out.
"""

import functools

import numpy as np

import concourse.bacc as bacc
import concourse.bass as bass
import concourse.tile as tile
from concourse import mybir

F32 = mybir.dt.float32
BF16 = mybir.dt.bfloat16
I16 = mybir.dt.int16


def cdiv(a, b):
    return -(-a // b)


# ---------------------------------------------------------------------------
# problem configuration (hardcoded for the graded problem)
# ---------------------------------------------------------------------------
N_CORES = 8
REAL_CFG = dict(B=128, BL=16, T=51, NPATCH=196, FEAT=2048, E=512, D=512, V=10000,
                bf16=True)


def derived(cfg):
    d = dict(cfg)
    BL, T = d["BL"], d["T"]
    d["G4"] = 4 * d["D"]
    d["ROWS"] = BL * T
    d["MCH"] = cdiv(d["ROWS"], 128)
    d["ROWS_PAD"] = d["MCH"] * 128
    d["KD"] = d["D"] // 128
    d["KE"] = d["E"] // 128
    d["KF"] = d["FEAT"] // 128
    d["BR"] = BL * d["NPATCH"]
    d["KB"] = cdiv(d["BR"], 128)
    d["NCH"] = cdiv(d["G4"], 512)
    d["NV"] = cdiv(d["V"], 512)
    d["S"] = cdiv(d["ROWS"], 16)
    return d


# ---------------------------------------------------------------------------
# kernel builder (shape-parameterized; one NeuronCore's program)
# ---------------------------------------------------------------------------
def build_decoder_kernel(tc, ins, outs, cfg, has_word_bias=False):
    nc = tc.nc
    c = derived(cfg)
    BL, T, NPATCH, FEAT, E, D, V = (
        c["BL"], c["T"], c["NPATCH"], c["FEAT"], c["E"], c["D"], c["V"])
    G4, ROWS, MCH, ROWS_PAD = c["G4"], c["ROWS"], c["MCH"], c["ROWS_PAD"]
    KD, KE, KF, BR, KB, NCH, NV = (
        c["KD"], c["KE"], c["KF"], c["BR"], c["KB"], c["NCH"], c["NV"])
    assert D % 128 == 0 and E % 128 == 0 and FEAT % 512 == 0 and G4 % 512 == 0
    assert BL <= 128 and (E * 4) % 256 == 0
    MT = BF16 if cfg.get("bf16") else F32  # matmul-operand dtype

    xw_dram = nc.dram_tensor("xw_scratch", [ROWS_PAD, G4], MT, kind="Internal").ap()

    import contextlib
    ctx = contextlib.ExitStack()
    with ctx:
        singles = ctx.enter_context(tc.tile_pool(name="singles", bufs=1))
        s8 = ctx.enter_context(tc.tile_pool(name="s8", bufs=3))
        xwp = ctx.enter_context(tc.tile_pool(name="xwp", bufs=3))
        actp = ctx.enter_context(tc.tile_pool(name="actp", bufs=2))
        cellp = ctx.enter_context(tc.tile_pool(name="cellp", bufs=2))
        outp = ctx.enter_context(tc.tile_pool(name="outp", bufs=4))
        psum = ctx.enter_context(tc.tile_pool(name="psum", bufs=1, space="PSUM"))
        pstr = ctx.enter_context(tc.tile_pool(name="pstr", bufs=2, space="PSUM"))
        psio = ctx.enter_context(tc.tile_pool(name="psio", bufs=2, space="PSUM"))

        # ---- persistent tiles -------------------------------------------
        ident_sb = singles.tile([128, 128], MT)
        nc.sync.dma_start(out=ident_sb, in_=ins["ident"])
        whh_sb = singles.tile([128, KD, G4], MT)
        nc.sync.dma_start(
            out=whh_sb, in_=ins["whhT"].rearrange("(k p) g -> p k g", p=128))
        biasg_sb = singles.tile([128, G4], F32)
        bg = ins["biasg"]
        nc.sync.dma_start(
            out=biasg_sb,
            in_=bass.AP(tensor=bg.tensor, offset=bg.offset,
                        ap=[[0, 128], bg.ap[-1]]))
        mask_sb = singles.tile([128, MCH], F32)
        nc.sync.dma_start(out=mask_sb, in_=ins["mask"])
        ones_sb = singles.tile([128, KB, BL], MT)
        nc.sync.dma_start(
            out=ones_sb, in_=ins["ones"].rearrange("(k p) b -> p k b", p=128))
        attb_sb = singles.tile([BL, D], F32)
        ab = ins["att1b"]
        nc.sync.dma_start(
            out=attb_sb,
            in_=bass.AP(tensor=ab.tensor, offset=ab.offset,
                        ap=[[0, BL], ab.ap[-1]]))
        idxs_sb = singles.tile([128, c["S"]], I16)
        nc.sync.dma_start(out=idxs_sb, in_=ins["idx"])
        hTh = singles.tile([128, KD, ROWS], MT)
        hT0 = singles.tile([128, KD, BL], MT)
        embT = singles.tile([128, KE, ROWS_PAD], MT)
        meanT = singles.tile([128, KF, BL], MT)
        emb_sb = singles.tile([128, MCH, E], MT)

        # ---- P1: mean over patches -> h0 --------------------------------
        ps_mean = psum.tile([BL, max(G4, FEAT)], F32, tag="gates")
        for kb in range(KB):
            rk = min(128, BR - kb * 128)
            ft = s8.tile([128, FEAT], MT, tag="s8", name="ft")
            nc.sync.dma_start(out=ft[:rk], in_=ins["feats"][kb * 128:kb * 128 + rk, :])
            for nf in range(FEAT // 512):
                nc.tensor.matmul(
                    ps_mean[:, nf * 512:(nf + 1) * 512],
                    lhsT=ones_sb[:rk, kb, :],
                    rhs=ft[:rk, nf * 512:(nf + 1) * 512],
                    start=(kb == 0), stop=(kb == KB - 1))
        mean_sb = actp.tile([BL, max(G4, FEAT)], MT, tag="act", name="mean_sb")
        nc.scalar.activation(
            mean_sb[:, :FEAT], ps_mean[:, :FEAT],
            mybir.ActivationFunctionType.Copy, scale=1.0 / NPATCH)
        # transpose mean -> meanT
        ps_mt = pstr.tile([128, KF, BL], MT, tag="tr", name="ps_mt")
        for kf in range(KF):
            nc.tensor.transpose(
                ps_mt[:, kf, :], mean_sb[:BL, kf * 128:(kf + 1) * 128],
                ident_sb[:BL, :BL])
        nc.scalar.copy(meanT, ps_mt)
        # h0 = mean @ att1T + att1b
        ps_h0 = psio.tile([128, 512], F32, tag="io", name="ps_h0")
        for kf in range(KF):
            at = s8.tile([128, D], MT, tag="s8", name="at")
            nc.sync.dma_start(
                out=at, in_=ins["att1T"].rearrange("(k p) d -> p k d", p=128)[:, kf, :])
            nc.tensor.matmul(ps_h0[:BL, :D], lhsT=meanT[:, kf, :], rhs=at,
                             start=(kf == 0), stop=(kf == KF - 1))
        h0_sb = cellp.tile([BL, D], MT, tag="h", name="h0_sb")
        nc.vector.tensor_add(h0_sb, ps_h0[:BL, :D], attb_sb)
        ps_t0 = pstr.tile([128, KD, BL], MT, tag="tr", name="ps_t0")
        for kd in range(KD):
            nc.tensor.transpose(
                ps_t0[:, kd, :], h0_sb[:, kd * 128:(kd + 1) * 128],
                ident_sb[:BL, :BL])
        nc.scalar.copy(hT0, ps_t0)

        # ---- P2: embedding gather + XW precompute -----------------------
        nc.vector.memset(emb_sb, 0.0)
        nc.gpsimd.dma_gather(
            out_ap=emb_sb, in_ap=ins["emb"], idxs_ap=idxs_sb,
            num_idxs=ROWS, num_idxs_reg=ROWS, elem_size=E)
        for m in range(MCH):
            for ke in range(KE):
                ps_e = pstr.tile([128, 128], MT, tag="tr", name="ps_e")
                nc.tensor.transpose(
                    ps_e, emb_sb[:, m, ke * 128:(ke + 1) * 128], ident_sb)
                if (m + ke) % 2 == 0:
                    nc.scalar.copy(embT[:, ke, m * 128:(m + 1) * 128], ps_e)
                else:
                    nc.vector.tensor_copy(embT[:, ke, m * 128:(m + 1) * 128], ps_e)
        for nch in range(NCH):
            gs = slice(nch * 512, (nch + 1) * 512)
            wi = s8.tile([128, KE, 512], MT, tag="s8", name="wi")
            nc.sync.dma_start(
                out=wi,
                in_=ins["wihT"].rearrange("(k p) g -> p k g", p=128)[:, :, gs])
            for m in range(MCH):
                mr = min(128, ROWS - m * 128)
                ps_xw = psio.tile([128, 512], F32, tag="io", name="ps_xw")
                for ke in range(KE):
                    nc.tensor.matmul(
                        ps_xw[:mr], lhsT=embT[:, ke, m * 128:m * 128 + mr],
                        rhs=wi[:, ke, :], start=(ke == 0), stop=(ke == KE - 1))
                xo = outp.tile([128, 512], MT, tag="xwout", name="xo")
                nc.vector.tensor_add(xo[:mr], ps_xw[:mr], biasg_sb[:mr, gs])
                nc.sync.dma_start(
                    out=xw_dram[m * 128:m * 128 + mr, gs], in_=xo[:mr])

        # ---- P3: LSTM recurrence ----------------------------------------
        c_prev = cellp.tile([BL, D], F32, tag="c", name="c_init")
        nc.vector.memset(c_prev, 0.0)
        h_prev = hT0
        for t in range(T):
            xw_sb = xwp.tile([BL, G4], MT, tag="xw", name="xw_sb")
            nc.sync.dma_start(out=xw_sb, in_=xw_dram[t * BL:(t + 1) * BL, :])
            ps_g = psum.tile([BL, max(G4, FEAT)], F32, tag="gates", name="ps_g")
            for nch in range(NCH):
                gs = slice(nch * 512, (nch + 1) * 512)
                nc.tensor.matmul(ps_g[:, gs], lhsT=ident_sb[:BL, :BL],
                                 rhs=xw_sb[:, gs], start=True, stop=False)
            for kd in range(KD):
                for nch in range(NCH):
                    gs = slice(nch * 512, (nch + 1) * 512)
                    nc.tensor.matmul(
                        ps_g[:, gs], lhsT=h_prev[:, kd, :], rhs=whh_sb[:, kd, gs],
                        start=False, stop=(kd == KD - 1))
            act_sb = actp.tile([BL, max(G4, FEAT)], F32, tag="act", name="act_sb")
            nc.scalar.activation(act_sb[:, 0:2 * D], ps_g[:, 0:2 * D],
                                 mybir.ActivationFunctionType.Sigmoid)
            nc.scalar.activation(act_sb[:, 2 * D:3 * D], ps_g[:, 2 * D:3 * D],
                                 mybir.ActivationFunctionType.Tanh)
            nc.scalar.activation(act_sb[:, 3 * D:4 * D], ps_g[:, 3 * D:4 * D],
                                 mybir.ActivationFunctionType.Sigmoid)
            ig = cellp.tile([BL, D], F32, tag="ig", name="ig")
            nc.vector.tensor_mul(ig, act_sb[:, 0:D], act_sb[:, 2 * D:3 * D])
            fc = cellp.tile([BL, D], F32, tag="fc", name="fc")
            nc.vector.tensor_mul(fc, act_sb[:, D:2 * D], c_prev)
            c_new = cellp.tile([BL, D], F32, tag="c", name="c_new")
            nc.vector.tensor_add(c_new, fc, ig)
            tanhc = cellp.tile([BL, D], F32, tag="th", name="tanhc")
            nc.scalar.activation(tanhc, c_new, mybir.ActivationFunctionType.Tanh)
            h_sb = cellp.tile([BL, D], MT, tag="h", name="h_sb")
            nc.vector.tensor_mul(h_sb, act_sb[:, 3 * D:4 * D], tanhc)
            ps_tr = pstr.tile([128, KD, BL], MT, tag="tr", name="ps_tr")
            for kd in range(KD):
                nc.tensor.transpose(
                    ps_tr[:, kd, :], h_sb[:, kd * 128:(kd + 1) * 128],
                    ident_sb[:BL, :BL])
            nc.scalar.copy(hTh[:, :, t * BL:(t + 1) * BL], ps_tr)
            c_prev = c_new
            h_prev = hTh[:, :, t * BL:(t + 1) * BL]

        # ---- P4: vocab projection + mask + store ------------------------
        for nv in range(NV):
            nvs = min(512, V - nv * 512)
            wv = s8.tile([128, KD, 512], MT, tag="s8", name="wv")
            nc.sync.dma_start(
                out=wv[:, :, :nvs],
                in_=ins["wordT"].rearrange("(k p) v -> p k v", p=128)
                [:, :, nv * 512:nv * 512 + nvs])
            if has_word_bias:
                wb = ins["wordb"]
                wb_sb = outp.tile([128, 512], F32, tag="wb", name="wb_sb")
                nc.sync.dma_start(
                    out=wb_sb[:, :nvs],
                    in_=bass.AP(tensor=wb.tensor,
                                offset=wb.offset + nv * 512,
                                ap=[[0, 128], [1, nvs]]))
            for m in range(MCH):
                mr = min(128, ROWS - m * 128)
                ps_w = psio.tile([128, 512], F32, tag="io", name="ps_w")
                for kd in range(KD):
                    nc.tensor.matmul(
                        ps_w[:mr, :nvs], lhsT=hTh[:, kd, m * 128:m * 128 + mr],
                        rhs=wv[:, kd, :nvs], start=(kd == 0), stop=(kd == KD - 1))
                ou = outp.tile([128, 512], F32, tag="out", name="ou")
                if has_word_bias:
                    nc.vector.tensor_add(ou[:mr, :nvs], ps_w[:mr, :nvs],
                                         wb_sb[:mr, :nvs])
                    nc.vector.tensor_scalar_mul(ou[:mr, :nvs], ou[:mr, :nvs],
                                                mask_sb[:mr, m:m + 1])
                elif (m + nv) % 2 == 0:
                    nc.scalar.activation(
                        ou[:mr, :nvs], ps_w[:mr, :nvs],
                        mybir.ActivationFunctionType.Copy,
                        scale=mask_sb[:mr, m:m + 1])
                else:
                    nc.vector.tensor_scalar_mul(ou[:mr, :nvs], ps_w[:mr, :nvs],
                                                mask_sb[:mr, m:m + 1])
                nc.sync.dma_start(
                    out=outs["preds"][m * 128:m * 128 + mr,
                                      nv * 512:nv * 512 + nvs],
                    in_=ou[:mr, :nvs])


# ---------------------------------------------------------------------------
# host-side input prep
# ---------------------------------------------------------------------------
def prep_core_inputs(cfg, rows_feats, rows_caps, rows_declen, shared):
    """Build the per-core in_map. rows_* are this core's (sorted) slices."""
    c = derived(cfg)
    BL, T = c["BL"], c["T"]
    m = dict(shared)
    mt = np.dtype(mybir.dt.np(BF16)) if cfg.get("bf16") else np.float32
    m["feats"] = np.ascontiguousarray(
        rows_feats.reshape(c["BR"], c["FEAT"]).astype(mt))
    idx_flat = np.ascontiguousarray(rows_caps[:, :T].T).reshape(-1)
    pad = c["S"] * 16 - idx_flat.shape[0]
    idx_flat = np.concatenate([idx_flat, -np.ones(pad, np.int64)])
    m["idx"] = np.ascontiguousarray(
        np.tile(idx_flat.reshape(c["S"], 16).T, (8, 1)).astype(np.int16))
    mk = (np.arange(T)[:, None] < rows_declen[None, :]).astype(np.float32)
    mk = np.concatenate([mk.reshape(-1),
                         np.zeros(c["ROWS_PAD"] - c["ROWS"], np.float32)])
    m["mask"] = np.ascontiguousarray(mk.reshape(c["MCH"], 128).T)
    return m


def prep_shared_inputs(cfg, emb_table, att1_w, att1_b, w_ih, w_hh, b_ih, b_hh,
                       word_w, word_b):
    c = derived(cfg)
    f32 = np.float32
    mt = np.dtype(mybir.dt.np(BF16)) if cfg.get("bf16") else f32
    shared = {
        "emb": np.ascontiguousarray(np.asarray(emb_table).astype(mt)),
        "wihT": np.ascontiguousarray(np.asarray(w_ih).T.astype(mt)),
        "whhT": np.ascontiguousarray(np.asarray(w_hh).T.astype(mt)),
        "biasg": (np.asarray(b_ih) + np.asarray(b_hh)).astype(f32),
        "att1T": np.ascontiguousarray(np.asarray(att1_w).T.astype(mt)),
        "att1b": np.ascontiguousarray(att1_b, f32),
        "wordT": np.ascontiguousarray(np.asarray(word_w).T.astype(mt)),
        "ident": np.eye(128).astype(mt),
    }
    ones = np.zeros((c["KB"] * 128, c["BL"]), f32).astype(mt)
    for b in range(c["BL"]):
        ones[b * c["NPATCH"]:(b + 1) * c["NPATCH"], b] = 1.0
    shared["ones"] = ones
    if np.any(word_b):
        shared["wordb"] = np.ascontiguousarray(word_b, f32)
    return shared


def declare_io(nc, cfg, has_word_bias):
    c = derived(cfg)
    MT = BF16 if cfg.get("bf16") else F32
    def di(name, shape, dt=F32):
        return nc.dram_tensor(name, list(shape), dt, kind="ExternalInput").ap()
    ins = {
        "feats": di("feats", (c["BR"], c["FEAT"]), MT),
        "emb": di("emb", (c["V"], c["E"]), MT),
        "idx": di("idx", (128, c["S"]), I16),
        "wihT": di("wihT", (c["E"], c["G4"]), MT),
        "whhT": di("whhT", (c["D"], c["G4"]), MT),
        "biasg": di("biasg", (c["G4"],)),
        "att1T": di("att1T", (c["FEAT"], c["D"]), MT),
        "att1b": di("att1b", (c["D"],)),
        "wordT": di("wordT", (c["D"], c["V"]), MT),
        "ident": di("ident", (128, 128), MT),
        "ones": di("ones", (c["KB"] * 128, c["BL"]), MT),
        "mask": di("mask", (128, c["MCH"])),
    }
    if has_word_bias:
        ins["wordb"] = di("wordb", (c["V"],))
    outs = {
        "preds": nc.dram_tensor(
            "preds", [c["ROWS"], c["V"]], F32, kind="ExternalOutput").ap(),
    }
    return ins, outs


@functools.lru_cache(maxsize=2)
def _get_compiled(has_word_bias):
    cfg = REAL_CFG
    nc = bacc.Bacc("TRN2", target_bir_lowering=False, debug=False,
                   enable_asserts=False, num_devices=N_CORES)
    ins, outs = declare_io(nc, cfg, has_word_bias)
    with tile.TileContext(nc) as tc:
        build_decoder_kernel(tc, ins, outs, cfg, has_word_bias)
    nc.compile()
    return nc


# ---------------------------------------------------------------------------
# public entry point
# ---------------------------------------------------------------------------
def kernel(image_features, encoded_captions, caption_lengths, emb_table,
           att1_w, att1_b, w_ih, w_hh, b_ih, b_hh, word_w, word_b,
           _trace=False, _trace_kwargs=None):
    from concourse import bass_utils

    cfg = REAL_CFG
    c = derived(cfg)
    B, BL, T, V = cfg["B"], cfg["BL"], cfg["T"], cfg["V"]

    lens = np.asarray(caption_lengths)[:, 0]
    sort_ind = np.argsort(-lens.astype(np.int64), kind="stable")
    lens_s = lens[sort_ind]
    caps = np.asarray(encoded_captions)[sort_ind]
    dec_len = (lens_s - 1).astype(caption_lengths.dtype)
    idx_dtype = np.int64 if caption_lengths.dtype == np.int64 else np.int32
    sort_ind = sort_ind.astype(idx_dtype)

    has_wb = bool(np.any(np.asarray(word_b)))
    shared = prep_shared_inputs(cfg, emb_table, att1_w, att1_b, w_ih, w_hh,
                                b_ih, b_hh, word_w, word_b)
    feats_np = np.asarray(image_features)
    in_maps = []
    for cid in range(N_CORES):
        rows = sort_ind[cid::N_CORES].astype(np.int64)
        in_maps.append(prep_core_inputs(
            cfg, feats_np[rows], caps[cid::N_CORES],
            np.asarray(dec_len)[cid::N_CORES].astype(np.int64), shared))

    nc = _get_compiled(has_wb)
    res = bass_utils.run_bass_kernel_spmd(
        nc, in_maps, list(range(N_CORES)), trace=_trace,
        **(_trace_kwargs or {}))

    predictions = np.empty((B, T, V), np.float32)
    for cid in range(N_CORES):
        p = res.results[cid]["preds"].reshape(T, BL, V)
        predictions[cid::N_CORES] = np.swapaxes(p, 0, 1)
    kernel.last_results = res
    return predictions, caps, dec_len, sort_ind
